# revision 1
# baseline (speedup 1.0000x reference)
"""BernConv (K=2) GNN message passing on 8 Trainium2 NeuronCores.

Self-contained kernel: kernel(**inputs) -> np.ndarray [N, 32] float32.

Strategy (v2, ap_gather): keep the whole fp16 feature table SBUF-resident
per source-chunk and gather src rows with the GPSIMD `ap_gather` SBUF
gather (the dma_gather/HBM path is descriptor-rate bound at ~60 GB/s
effective for 256B random reads).

Layout: nodes dealt (by degree) over 8 cores x 8 GPSIMD groups x SLOTS
slots; feature dim pairs (2l, 2l+1) live on partition lane l of each
16-partition group, i.e. every group holds all 32 dims at d=2 fp16 per
node. The gather table [128, NLOC*2] fp16 (row c*16+l = core c, lane l)
is AllGathered between rounds; each round every core streams the table
in NCHUNK=4 source chunks into a [128, CH*2] SBUF tile (8 group
replicas) and runs per-group ap_gathers + bucketed DVE segment reduces:
    g0 = dh*feat;  g_k = g_{k-1} + dh^2 * agg(g_{k-1})   (k = 1, 2)
    out = s1 * dh^-1 * g2 - s2 * dh * agg(g2)
"""
import sys
sys.path.insert(0, "/opt/trn_rl_repo")

import numpy as np
import concourse.bacc as bacc
import concourse.mybir as mybir
import concourse.tile as tile
from concourse import bass_utils

NC = 8
G = 8
P = 128
D = 32
SLOTS = 784              # last slot (783) reserved as guaranteed-zero
NLOC = G * SLOTS         # 6272
NPAD = NC * NLOC         # 50176
NCHUNK = 4
CH = NPAD // NCHUNK      # 12544
ZIDX = CH - 1            # rel row of core 2k+1, g=7, s=783 -> reserved zero
JMAX_RAW = 2560          # target segment size (pre %16 pad)

F32 = mybir.dt.float32
F16 = mybir.dt.float16
I16 = mybir.dt.int16


# --------------------------------------------------------------------------
# host-side layout
# --------------------------------------------------------------------------

def choose_levels(req, max_levels=14):
    """Bucket levels minimizing total padded degree (DP on req histogram)."""
    Lmax = int(req.max())
    hist = np.bincount(req, minlength=Lmax + 1).astype(np.int64)
    cnt_le = hist.cumsum()
    INF = float("inf")

    def interval_cost(a, b):
        return int(cnt_le[b] - cnt_le[a]) * b

    f = np.full((max_levels + 1, Lmax + 1), INF)
    prev = np.zeros((max_levels + 1, Lmax + 1), dtype=np.int64)
    f[0, 0] = 0.0
    for m in range(1, max_levels + 1):
        for b in range(1, Lmax + 1):
            best, besta = INF, 0
            for a in range(0, b):
                if f[m - 1, a] == INF:
                    continue
                c = f[m - 1, a] + interval_cost(a, b)
                if c < best:
                    best, besta = c, a
            f[m, b] = best
            prev[m, b] = besta
    m_best = int(np.argmin(f[:, Lmax]))
    levels = []
    b, m = Lmax, m_best
    while b > 0:
        levels.append(b)
        b = int(prev[m, b])
        m -= 1
    return np.array(sorted(levels), dtype=np.int64)


def balance_assign(src, dst, deg, n_nodes):
    """Assign nodes to the 64 (c,g) groups, batch-dealing by degree (keeps
    group sizes and degree profiles aligned) while greedily choosing each
    node's CHUNK to flatten its out-neighbors' per-chunk in-edge counts
    (cuts the bucket padding that is driven by max-over-chunks)."""
    order = np.argsort(-deg, kind="stable")
    # out-adjacency sorted by src
    eo = np.argsort(src, kind="stable")
    s_src = src[eo]
    s_dst = dst[eo]
    out_start = np.searchsorted(s_src, np.arange(n_nodes))
    out_end = np.searchsorted(s_src, np.arange(n_nodes), side="right")

    cnt = np.zeros((n_nodes, NCHUNK), dtype=np.int32)
    curmax = np.zeros(n_nodes, dtype=np.int32)
    node_cg = np.full(n_nodes, -1, dtype=np.int64)
    gpc = G * (NC // NCHUNK)          # groups per chunk (16)
    nb = (n_nodes + 63) // 64

    def sweep():
        # rotate group fill order per batch so profiles stay aligned
        for b in range(nb):
            batch = order[b * 64:(b + 1) * 64]
            nbrs = [s_dst[out_start[i]:out_end[i]] for i in batch]
            # remove current contribution (refinement passes)
            for j, i in enumerate(batch):
                if node_cg[i] >= 0:
                    k_old = (node_cg[i] // G) // (NC // NCHUNK)
                    chunk_total[k_old] -= 1
                    if len(nbrs[j]):
                        np.add.at(cnt, (nbrs[j], k_old), -1)
                        curmax[nbrs[j]] = cnt[nbrs[j]].max(axis=1)
            costs = np.zeros((len(batch), NCHUNK), dtype=np.int64)
            for j, nb_j in enumerate(nbrs):
                if len(nb_j):
                    m = curmax[nb_j]
                    c = cnt[nb_j]
                    costs[j] = (2 * np.maximum(c + 1 - m[:, None], 0)
                                + (c + 1 == m[:, None])).sum(axis=0)
            # soft per-batch quota (2x the fair share) + hard global cap
            cap = np.full(NCHUNK, 2 * gpc, dtype=np.int64)
            slot_in_chunk = np.zeros(NCHUNK, dtype=np.int64)
            sc = np.sort(costs, axis=1)
            regret = sc[:, 1] - sc[:, 0] if NCHUNK > 1 else sc[:, 0]
            for j in np.argsort(-regret):
                ks = np.argsort(costs[j], kind="stable")
                k = next(k for k in ks
                         if cap[k] > 0 and chunk_total[k] < CHCAP)
                cap[k] -= 1
                chunk_total[k] += 1
                i = batch[j]
                g_off = (b + slot_in_chunk[k]) % gpc
                node_cg[i] = (k * (NC // NCHUNK) + g_off // G) * G + (g_off % G)
                slot_in_chunk[k] += 1
                nb_j = nbrs[j]
                if len(nb_j):
                    np.add.at(cnt, (nb_j, k), 1)
                    curmax[nb_j] = np.maximum(curmax[nb_j], cnt[nb_j, k])

    CHCAP = gpc * (SLOTS - 1) - 16
    chunk_total = np.zeros(NCHUNK, dtype=np.int64)
    for _ in range(5):
        sweep()
    return node_cg


def build_layout(src, dst, n_nodes):
    E = src.shape[0]
    deg = np.bincount(dst, minlength=n_nodes).astype(np.int64)

    node_cg = balance_assign(src, dst, deg, n_nodes)
    node_core = node_cg // G
    node_grp = node_cg % G
    node_chunk = node_core // (NC // NCHUNK)   # chunk of a node as src

    cnt = np.zeros((n_nodes, NCHUNK), dtype=np.int64)
    np.add.at(cnt, (dst, node_chunk[src]), 1)
    req = np.maximum(cnt.max(axis=1), 1)

    levels = choose_levels(req)
    ghat = levels[np.searchsorted(levels, req)]

    # re-deal within each chunk by ghat rank: the 16 groups of a chunk get
    # exactly aligned level profiles, so the shared template (max over all
    # 64 groups) is just the elementwise max of the 4 chunk profiles.
    gpc = G * (NC // NCHUNK)
    node_core = np.empty(n_nodes, dtype=np.int64)
    node_grp = np.empty(n_nodes, dtype=np.int64)
    node_slot = np.empty(n_nodes, dtype=np.int64)
    tmpl = np.zeros(SLOTS - 1, dtype=np.int64)
    for kchunk in range(NCHUNK):
        nk = np.where(node_chunk == kchunk)[0]
        nk = nk[np.lexsort((nk, -ghat[nk]))]
        r = np.arange(len(nk))
        go = r % gpc
        node_core[nk] = 2 * kchunk + go // G
        node_grp[nk] = go % G
        node_slot[nk] = r // gpc
        prof = ghat[nk][::gpc]            # rank-0-of-each-slot = profile max
        assert len(prof) <= SLOTS - 1, len(prof)
        tmpl[: len(prof)] = np.maximum(tmpl[: len(prof)], prof)
    node_cg = node_core * G + node_grp
    nslots_used = int((tmpl > 0).sum())
    tmpl = tmpl[:nslots_used]
    prefixB = np.concatenate([[0], np.cumsum(tmpl)])
    TJ = int(prefixB[-1])

    # segments: greedy cut at slot boundaries, each <= JMAX_RAW raw idxs
    seg_bounds = [0]
    for s in range(nslots_used):
        if prefixB[s + 1] - prefixB[seg_bounds[-1]] > JMAX_RAW:
            seg_bounds.append(s)
    seg_bounds.append(nslots_used)
    segs = []
    for i in range(len(seg_bounds) - 1):
        lo, hi = seg_bounds[i], seg_bounds[i + 1]
        raw = int(prefixB[hi] - prefixB[lo])
        # %32 (not %16): keeps every idx-slice base 4-byte aligned — the
        # gather ucode reads idxs as 32-bit words and a 2-byte-misaligned
        # AP base corrupts every 4th word's high half.
        pad = (-raw) % 32
        segs.append((lo, hi, raw, raw + pad))
    JMAXP = max(s[3] for s in segs)
    TPJ = sum(s[3] for s in segs)

    # runs of equal level inside each segment
    runs = []
    for si, (lo, hi, raw, padded) in enumerate(segs):
        s = lo
        off = 0
        while s < hi:
            b = int(tmpl[s])
            e = s
            while e < hi and tmpl[e] == b:
                e += 1
            runs.append((si, off, e - s, b, s))
            off += (e - s) * b
            s = e

    # fill per (c,g,chunk) idx arrays with ZIDX pads
    pi = node_core * NLOC + node_grp * SLOTS + node_slot
    src_row = pi[src]
    e_chunk = src_row // CH
    e_rel = src_row - e_chunk * CH

    okey_e = np.lexsort((e_rel, e_chunk, dst))
    s_dst = dst[okey_e]
    s_chunk = e_chunk[okey_e]
    s_rel = e_rel[okey_e]
    key = s_dst * NCHUNK + s_chunk
    run_start = np.searchsorted(key, key)
    erank = np.arange(E) - run_start
    assert (erank < ghat[s_dst]).all()

    A = np.full((NC * G, NCHUNK, TJ), ZIDX, dtype=np.int16)
    flat_pos = (node_cg[s_dst] * NCHUNK + s_chunk) * TJ \
        + prefixB[node_slot[s_dst]] + erank
    A.reshape(-1)[flat_pos] = s_rel.astype(np.int16)

    parts = []
    for (lo, hi, raw, padded) in segs:
        blk = A[:, :, prefixB[lo]:prefixB[hi]]
        if padded > raw:
            padblk = np.full((NC * G, NCHUNK, padded - raw), ZIDX, np.int16)
            blk = np.concatenate([blk, padblk], axis=2)
        parts.append(blk)
    AP_ = np.concatenate(parts, axis=2)

    return dict(
        deg=deg, node_cg=node_cg, node_core=node_core, node_grp=node_grp,
        node_slot=node_slot, pi=pi, tmpl=tmpl, prefixB=prefixB,
        segs=segs, runs=runs, TJ=TJ, TPJ=TPJ, JMAXP=JMAXP,
        nslots_used=nslots_used, idx=AP_, levels=levels,
    )


def make_host_data(feat, weight, src, dst):
    n_nodes = feat.shape[0]
    lay = build_layout(src, dst, n_nodes)

    deg = lay["deg"].astype(np.float64)
    degc = np.maximum(deg, 1.0).astype(np.float32)
    dh = (degc ** -0.5).astype(np.float32)
    dh2 = dh * dh
    dhinv = 1.0 / dh

    node_core = lay["node_core"]
    node_grp = lay["node_grp"]
    node_slot = lay["node_slot"]
    nodes = np.arange(n_nodes)

    def slot_tile(vals):
        t = np.zeros((NC, P, SLOTS), np.float32)
        for l in range(16):
            t[node_core, node_grp * 16 + l, node_slot] = vals
        return t

    dh2l = slot_tile(dh2)
    dhhl = slot_tile(dh)
    dhil = slot_tile(dhinv)

    g0v = feat * dh[:, None]
    g0slot = np.zeros((NC, P, SLOTS, 2), np.float32)
    g0tab = np.zeros((P, NLOC, 2), np.float16)
    for l in range(16):
        g0slot[node_core, node_grp * 16 + l, node_slot, 0] = g0v[nodes, 2 * l]
        g0slot[node_core, node_grp * 16 + l, node_slot, 1] = g0v[nodes, 2 * l + 1]
        g0tab[node_core * 16 + l, node_grp * SLOTS + node_slot, 0] = \
            g0v[nodes, 2 * l].astype(np.float16)
        g0tab[node_core * 16 + l, node_grp * SLOTS + node_slot, 1] = \
            g0v[nodes, 2 * l + 1].astype(np.float16)
    g0slot = g0slot.reshape(NC, P, SLOTS * 2)
    g0tab = g0tab.reshape(P, NLOC * 2)
    # pre-replicated round-1 chunk tables (input upload is free)
    g0rep = []
    for kc in range(NCHUNK):
        rows = g0tab[2 * kc * 16:(2 * kc + 2) * 16].reshape(2, 16, NLOC * 2)
        blk = np.concatenate([rows[0], rows[1]], axis=1)      # [16, CH*2]
        g0rep.append(np.ascontiguousarray(np.tile(blk, (G, 1))))

    idx = lay["idx"].reshape(NC, G, NCHUNK * lay["TPJ"])
    idxw = idx.reshape(NC, G, -1, 16).transpose(0, 1, 3, 2)
    idxw = np.ascontiguousarray(idxw.reshape(NC, P, -1))

    coef = np.array([[0.25, 0.5, 0.25, 0.0, 0.5, 0.25]], np.float32)
    w2 = np.ascontiguousarray(weight.reshape(1, 3).astype(np.float32))

    per_core = []
    for c in range(NC):
        pc = dict(
            g0tab=g0tab, g0slot=np.ascontiguousarray(g0slot[c]),
            idx=idxw[c], dh2l=np.ascontiguousarray(dh2l[c]),
            dhhl=np.ascontiguousarray(dhhl[c]),
            dhil=np.ascontiguousarray(dhil[c]),
            w=w2, coef=coef,
        )
        for kc in range(NCHUNK):
            pc[f"g0rep{kc}"] = g0rep[kc]
        per_core.append(pc)
    return lay, per_core


def assemble_output(lay, outs, n_nodes):
    full = np.stack(outs)                          # [NC, 128, SLOTS*2]
    full = full.reshape(NC, G, 16, SLOTS, 2)
    res = np.empty((n_nodes, D), np.float32)
    nc_, ng, ns = lay["node_core"], lay["node_grp"], lay["node_slot"]
    for l in range(16):
        res[:, 2 * l] = full[nc_, ng, l, ns, 0]
        res[:, 2 * l + 1] = full[nc_, ng, l, ns, 1]
    return res


# --------------------------------------------------------------------------
# device kernel
# --------------------------------------------------------------------------

def build_kernel(segs, runs, TPJ, JMAXP):
    TOTC = NCHUNK * TPJ // 16     # idx cols per partition
    S2 = SLOTS * 2

    nc = bacc.Bacc("TRN2", target_bir_lowering=False)
    g0rep_d = [nc.dram_tensor(f"g0rep{kc}", [P, CH * 2], F16,
                              kind="ExternalInput") for kc in range(NCHUNK)]
    g0slot_d = nc.dram_tensor("g0slot", [P, S2], F32, kind="ExternalInput")
    idx_d = nc.dram_tensor("idx", [P, TOTC], I16, kind="ExternalInput")
    dh2l_d = nc.dram_tensor("dh2l", [P, SLOTS], F32, kind="ExternalInput")
    dhhl_d = nc.dram_tensor("dhhl", [P, SLOTS], F32, kind="ExternalInput")
    dhil_d = nc.dram_tensor("dhil", [P, SLOTS], F32, kind="ExternalInput")
    w_d = nc.dram_tensor("w", [1, 3], F32, kind="ExternalInput")
    coef_d = nc.dram_tensor("coef", [1, 6], F32, kind="ExternalInput")
    out_d = nc.dram_tensor("out", [P, S2], F32, kind="ExternalOutput")

    seg_off = [0]
    for s in segs:
        seg_off.append(seg_off[-1] + s[3])

    with tile.TileContext(nc) as tc:
        with (
            tc.tile_pool(name="dram", bufs=1, space="DRAM") as dramp,
            tc.tile_pool(name="persist", bufs=1) as persist,
            tc.tile_pool(name="tblp", bufs=2) as tblp,
            tc.tile_pool(name="slabp", bufs=3) as slabp,
        ):
            gtabs = [dramp.tile([P, NLOC * 2], F16, name=f"gtab{k}",
                                addr_space="Shared")
                     for k in range(2)]
            bounces = [dramp.tile([16, NLOC * 2], F16, name=f"bnc{k}")
                       for k in range(2)]

            idxt = persist.tile([P, TOTC], I16, name="idxt")
            nc.sync.dma_start(out=idxt[:], in_=idx_d[:])
            dh2l = persist.tile([P, SLOTS], F32, name="dh2l")
            dhhl = persist.tile([P, SLOTS], F32, name="dhhl")
            dhil = persist.tile([P, SLOTS], F32, name="dhil")
            nc.sync.dma_start(out=dh2l[:], in_=dh2l_d[:])
            nc.sync.dma_start(out=dhhl[:], in_=dhhl_d[:])
            nc.sync.dma_start(out=dhil[:], in_=dhil_d[:])

            # scalars s1, s2
            wt = persist.tile([1, 3], F32, name="wt")
            coefs = persist.tile([1, 6], F32, name="coefs")
            nc.sync.dma_start(out=wt[:], in_=w_d[:])
            nc.sync.dma_start(out=coefs[:], in_=coef_d[:])
            wr = persist.tile([1, 3], F32, name="wr")
            nc.vector.tensor_scalar(out=wr[:], in0=wt[:], scalar1=0.0,
                                    scalar2=None, op0=mybir.AluOpType.max)
            sprod = persist.tile([1, 6], F32, name="sprod")
            nc.vector.tensor_tensor(out=sprod[:, 0:3], in0=wr[:],
                                    in1=coefs[:, 0:3], op=mybir.AluOpType.mult)
            nc.vector.tensor_tensor(out=sprod[:, 3:6], in0=wr[:],
                                    in1=coefs[:, 3:6], op=mybir.AluOpType.mult)
            svals = persist.tile([1, 2], F32, name="svals")
            nc.vector.tensor_reduce(out=svals[:, 0:1], in_=sprod[:, 0:3],
                                    axis=mybir.AxisListType.X,
                                    op=mybir.AluOpType.add)
            nc.vector.tensor_reduce(out=svals[:, 1:2], in_=sprod[:, 3:6],
                                    axis=mybir.AxisListType.X,
                                    op=mybir.AluOpType.add)
            sbc = persist.tile([P, 2], F32, name="sbc")
            nc.gpsimd.partition_broadcast(sbc[:], svals[:])
            # fold s1/s2 into the final-combine scale tiles up front so the
            # sliced final round needs no tensor_scalar (2-port) ops
            nc.vector.tensor_scalar(out=dhil[:], in0=dhil[:],
                                    scalar1=sbc[:, 0:1], scalar2=None,
                                    op0=mybir.AluOpType.mult)
            nc.vector.tensor_scalar(out=dhhl[:], in0=dhhl[:],
                                    scalar1=sbc[:, 1:2], scalar2=None,
                                    op0=mybir.AluOpType.mult)

            zt = persist.tile([P, 1], F32, name="zt")
            nc.gpsimd.memset(zt[:], 0.0)

            gA = persist.tile([P, S2], F32, name="gA")
            gB = persist.tile([P, S2], F32, name="gB")
            nc.sync.dma_start(out=gA[:], in_=g0slot_d[:])

            partials = [persist.tile([P, S2], F32, name=f"part{kc}")
                        for kc in range(NCHUNK)]
            for t in partials:
                nc.gpsimd.memset(t[:], 0.0)

            agg = persist.tile([P, S2], F32, name="agg")
            ut = persist.tile([P, S2], F32, name="ut")
            tt = persist.tile([P, S2], F32, name="tt")
            shipt = [persist.tile([P, S2], F16, name=f"shipt{i}")
                     for i in range(2)]

            for k in (1, 2, 3):
                slab0 = None
                for kc in range(NCHUNK):
                    tbl = tblp.tile([P, CH * 2], F16, tag="tbl",
                                    name=f"tbl{k}_{kc}")
                    if kc == 1 and slab0 is not None:
                        # order-pin: chunk-1 loads WAW on this corner, which
                        # depends on the chunk-0 first gather — stops the
                        # scheduler enqueueing c1 loads before that gather's
                        # sem waits (which use cumulative lane thresholds)
                        nc.vector.tensor_tensor(
                            out=tbl[:, 0:2], in0=slab0[:, 0:2],
                            in1=slab0[:, 0:2], op=mybir.AluOpType.mult)
                    if k == 1:
                        # round 1: host pre-replicated table, one flat DMA
                        nc.sync.dma_start(out=tbl[:], in_=g0rep_d[kc][:])
                    else:
                        tabsrc = gtabs[k - 2]
                        src32 = tabsrc[2 * kc * 16:(2 * kc + 2) * 16, :] \
                            .rearrange("(h l) f -> l h f", h=2)
                        for g in range(G):
                            nc.sync.dma_start(
                                out=tbl[g * 16:(g + 1) * 16, :].rearrange(
                                    "l (h f) -> l h f", h=2),
                                in_=src32)
                    for si, (lo, hi, raw, padded) in enumerate(segs):
                        slab = slabp.tile([P, JMAXP * 2], F16, tag="slab",
                                          name=f"slab{k}_{kc}_{si}")
                        if kc == 0 and si == 0:
                            slab0 = slab
                        base16 = (kc * TPJ + seg_off[si]) // 16
                        nc.gpsimd.ap_gather(
                            out_ap=slab[:, 0:padded * 2].rearrange(
                                "p (n d) -> p n d", d=2),
                            in_ap=tbl[:].rearrange("p (n d) -> p n d", d=2),
                            idxs_ap=idxt[:, base16:base16 + padded // 16],
                            channels=P, num_elems=CH, d=2, num_idxs=padded)
                        for (si2, off, R, b, s0) in runs:
                            if si2 != si:
                                continue
                            nc.vector.tensor_reduce(
                                out=partials[kc][:, s0 * 2:(s0 + R) * 2]
                                .rearrange("p (r j) -> p r j", j=2),
                                in_=slab[:, off * 2:(off + R * b) * 2]
                                .rearrange("p (r b j) -> p r j b", b=b, j=2),
                                axis=mybir.AxisListType.X,
                                op=mybir.AluOpType.add)
                        if kc == NCHUNK - 1:
                            # slot-sliced round tail overlaps the remaining
                            # gathers of the last chunk
                            gold = gA if k != 2 else gB
                            gnew = gB if k == 1 else gA
                            lo2 = lo * 2
                            hi2 = hi * 2 if si < len(segs) - 1 else S2
                            ns = (hi2 - lo2) // 2
                            nc.vector.tensor_tensor(
                                out=agg[:, lo2:hi2],
                                in0=partials[0][:, lo2:hi2],
                                in1=partials[1][:, lo2:hi2],
                                op=mybir.AluOpType.add)
                            nc.vector.tensor_tensor(
                                out=agg[:, lo2:hi2], in0=agg[:, lo2:hi2],
                                in1=partials[2][:, lo2:hi2],
                                op=mybir.AluOpType.add)
                            nc.vector.tensor_tensor(
                                out=agg[:, lo2:hi2], in0=agg[:, lo2:hi2],
                                in1=partials[3][:, lo2:hi2],
                                op=mybir.AluOpType.add)
                            if k < 3:
                                nc.vector.tensor_tensor(
                                    out=tt[:, lo2:hi2].rearrange(
                                        "p (s j) -> p s j", j=2),
                                    in0=agg[:, lo2:hi2].rearrange(
                                        "p (s j) -> p s j", j=2),
                                    in1=dh2l[:, lo2 // 2:hi2 // 2]
                                    .to_broadcast([P, ns, 2]),
                                    op=mybir.AluOpType.mult)
                                nc.vector.tensor_tensor(
                                    out=gnew[:, lo2:hi2], in0=tt[:, lo2:hi2],
                                    in1=gold[:, lo2:hi2],
                                    op=mybir.AluOpType.add)
                                sh = shipt[k - 1]
                                nc.vector.tensor_tensor(
                                    out=sh[:, lo2:hi2], in0=gnew[:, lo2:hi2],
                                    in1=zt[:].to_broadcast([P, hi2 - lo2]),
                                    op=mybir.AluOpType.add)
                                for g in range(G):
                                    nc.sync.dma_start(
                                        out=bounces[k - 1][
                                            :, g * S2 + lo2:g * S2 + hi2],
                                        in_=sh[g * 16:(g + 1) * 16, lo2:hi2])
                            else:
                                # out = s1*dhil*g2 - s2*dhhl*agg (s1, s2
                                # pre-folded into dhil/dhhl)
                                nc.vector.tensor_tensor(
                                    out=ut[:, lo2:hi2].rearrange(
                                        "p (s j) -> p s j", j=2),
                                    in0=gold[:, lo2:hi2].rearrange(
                                        "p (s j) -> p s j", j=2),
                                    in1=dhil[:, lo2 // 2:hi2 // 2]
                                    .to_broadcast([P, ns, 2]),
                                    op=mybir.AluOpType.mult)
                                nc.vector.tensor_tensor(
                                    out=tt[:, lo2:hi2].rearrange(
                                        "p (s j) -> p s j", j=2),
                                    in0=agg[:, lo2:hi2].rearrange(
                                        "p (s j) -> p s j", j=2),
                                    in1=dhhl[:, lo2 // 2:hi2 // 2]
                                    .to_broadcast([P, ns, 2]),
                                    op=mybir.AluOpType.mult)
                                nc.vector.tensor_tensor(
                                    out=ut[:, lo2:hi2], in0=ut[:, lo2:hi2],
                                    in1=tt[:, lo2:hi2],
                                    op=mybir.AluOpType.subtract)
                                nc.sync.dma_start(out=out_d[:, lo2:hi2],
                                                  in_=ut[:, lo2:hi2])

                if k < 3:
                    nc.gpsimd.collective_compute(
                        "AllGather", mybir.AluOpType.bypass,
                        replica_groups=[list(range(NC))],
                        ins=[bounces[k - 1].opt()], outs=[gtabs[k - 1].opt()])
    nc.compile()
    return nc


_CACHE = {}


def kernel(feat, weight, src, dst):
    feat = np.ascontiguousarray(np.asarray(feat, dtype=np.float32))
    weight = np.ascontiguousarray(np.asarray(weight, dtype=np.float32))
    src64 = np.asarray(src).astype(np.int64)
    dst64 = np.asarray(dst).astype(np.int64)
    n_nodes = feat.shape[0]

    lay, per_core = make_host_data(feat, weight, src64, dst64)
    key = (tuple(lay["segs"]), tuple(lay["runs"]), lay["TPJ"], lay["JMAXP"])
    if key not in _CACHE:
        _CACHE[key] = build_kernel(lay["segs"], lay["runs"], lay["TPJ"],
                                   lay["JMAXP"])
    nc = _CACHE[key]

    keys = ["g0slot", "idx", "dh2l", "dhhl", "dhil", "w", "coef"] + \
        [f"g0rep{kc}" for kc in range(NCHUNK)]
    in_maps = [{k: pc[k] for k in keys} for pc in per_core]
    res = bass_utils.run_bass_kernel_spmd(nc, in_maps, core_ids=list(range(NC)))
    outs = [res.results[c]["out"] for c in range(NC)]
    return assemble_output(lay, outs, n_nodes)



# revision 2
# speedup vs baseline: 1.0184x; 1.0184x over previous
"""BernConv (K=2) GNN message passing on 8 Trainium2 NeuronCores.

v3: SWDGE dma_gather SpMM. The node table lives in DRAM as 256B-stride rows
(32 fp16 dims in the first 64B). Each round, every core gathers its in-edge
src rows with dma_gather on 4 concurrent SWDGE queues (Q7 pairs emit
descriptors, the 16 SDMA engines move 64B/edge), then DVE bucket-reduces the
[128, cols, 32] fp16 slab into per-node sums.

Layout: 51200 padded nodes = 8 cores x 128 partitions x 50 slots. Nodes are
dealt to slots by global degree rank (band b -> slot b), so one shared
slot-size template works for all 1024 partition-streams. Table row
n = core*6400 + slot*128 + p. int16 gather idxs force a 2-phase split of the
table (rows < 32768 = phase A); a quota-constrained greedy 2-coloring picks
each node's region to balance every dst's A/B in-edge counts, and each
bucket is [A-run | B-run] with per-band level-padded sizes.

Recurrence in scaled space g = dh*x:
    g_k = g_{k-1} + dh^2 * agg(g_{k-1})   (k = 1, 2)
    out = s1 * dh^-1 * g2 - s2 * dh * agg(g2)
"""
import sys
sys.path.insert(0, "/opt/trn_rl_repo")

import numpy as np
import concourse.bacc as bacc
import concourse.mybir as mybir
import concourse.tile as tile
from concourse import bass_utils

NC = 8
P = 128
D = 32
SLOTS = 50
CPN = P * SLOTS          # nodes per core = 6400
NPAD = NC * CPN          # 51200
ALO = 32768              # phase-A rows
MAXCOL = 126             # cols per gather instr (16128 idxs, scratch limit)

F32 = mybir.dt.float32
F16 = mybir.dt.float16
I16 = mybir.dt.int16


# --------------------------------------------------------------------------
# host-side layout
# --------------------------------------------------------------------------

def choose_levels(vals, max_levels=16):
    """Quantize the positive values in `vals` up to <=max_levels levels,
    minimizing total padded sum (same DP as v2, equal weights)."""
    req = vals[vals > 0]
    if len(req) == 0:
        return np.array([], dtype=np.int64)
    Lmax = int(req.max())
    hist = np.bincount(req, minlength=Lmax + 1).astype(np.int64)
    cnt_le = hist.cumsum()
    INF = float("inf")
    f = np.full((max_levels + 1, Lmax + 1), INF)
    prev = np.zeros((max_levels + 1, Lmax + 1), dtype=np.int64)
    f[0, 0] = 0.0
    for m in range(1, max_levels + 1):
        for b in range(1, Lmax + 1):
            best, besta = INF, 0
            for a in range(0, b):
                if f[m - 1, a] == INF:
                    continue
                c = f[m - 1, a] + int(cnt_le[b] - cnt_le[a]) * b
                if c < best:
                    best, besta = c, a
            f[m, b] = best
            prev[m, b] = besta
    m_best = int(np.argmin(f[:, Lmax]))
    levels = []
    b, m = Lmax, m_best
    while b > 0:
        levels.append(b)
        b = int(prev[m, b])
        m -= 1
    return np.array(sorted(levels), dtype=np.int64)


def color_nodes(src, dst, band, out_start, out_end, s_dst, sweeps=12):
    """Greedy quota-constrained 2-coloring with refinement sweeps: assign
    each node lo (phase A) or hi, balancing every dst's A-count toward
    alpha*deg."""
    alpha = ALO / NPAD
    dev = np.zeros(NPAD, dtype=np.float64)
    is_lo = np.zeros(NPAD, dtype=bool)
    assigned = np.zeros(NPAD, dtype=bool)
    band_members = []
    for b in range(SLOTS):
        members = np.where(band == b)[0]
        odeg = out_end[members] - out_start[members]
        band_members.append(members[np.argsort(-odeg, kind="stable")])

    # initial greedy pass (linear score, incremental dev)
    for b in range(SLOTS):
        members = band_members[b]
        q_lo = 768 if b < 6 else 640
        q_hi = len(members) - q_lo
        for n in members:
            nbrs = s_dst[out_start[n]:out_end[n]]
            s = dev[nbrs].sum() if len(nbrs) else 0.0
            lo = (s <= 0 and q_lo > 0) or q_hi == 0
            assigned[n] = True
            is_lo[n] = lo
            if lo:
                q_lo -= 1
                if len(nbrs):
                    dev[nbrs] += 1.0 - alpha
            else:
                q_hi -= 1
                if len(nbrs):
                    dev[nbrs] -= alpha

    # pairwise-exchange refinement (cosh outlier penalty, annealed beta)
    def phi(x, beta):
        return np.cosh(np.clip(beta * x, -25, 25))

    for beta in [0.5, 1.0, 1.5, 2.0, 2.0, 2.5, 2.5, 3.0, 3.0, 3.5, 4.0][:max(sweeps - 1, 0)]:
        for b in range(SLOTS):
            members = band_members[b]
            segs = [s_dst[out_start[n]:out_end[n]] for n in members]
            lens = np.array([len(x) for x in segs])
            flat = np.concatenate([x for x in segs if len(x)]) \
                if lens.sum() else np.array([], dtype=np.int64)
            starts = np.concatenate([[0], np.cumsum(lens)[:-1]])
            dv = dev[flat]
            p0 = phi(dv, beta)
            gain_hi = np.zeros(len(members))   # gain of flipping lo -> hi
            gain_lo = np.zeros(len(members))   # gain of flipping hi -> lo
            if len(flat):
                d_hi = p0 - phi(dv - 1.0, beta)
                d_lo = p0 - phi(dv + 1.0, beta)
                nz = lens > 0
                sums_hi = np.add.reduceat(d_hi, starts[nz])
                sums_lo = np.add.reduceat(d_lo, starts[nz])
                gain_hi[nz] = sums_hi
                gain_lo[nz] = sums_lo
            lo_idx = np.where(is_lo[members])[0]
            hi_idx = np.where(~is_lo[members])[0]
            lo_order = lo_idx[np.argsort(-gain_hi[lo_idx])]
            hi_order = hi_idx[np.argsort(-gain_lo[hi_idx])]
            for i in range(min(len(lo_order), len(hi_order), 512)):
                a, h = lo_order[i], hi_order[i]
                if gain_hi[a] + gain_lo[h] <= 1e-9:
                    break
                na, nh = members[a], members[h]
                is_lo[na], is_lo[nh] = False, True
                if lens[a]:
                    dev[segs[a]] -= 1.0
                if lens[h]:
                    dev[segs[h]] += 1.0
    return is_lo


def build_layout(src, dst, n_nodes):
    E = src.shape[0]
    deg = np.bincount(dst, minlength=n_nodes).astype(np.int64)
    degp = np.concatenate([deg, np.zeros(NPAD - n_nodes, dtype=np.int64)])

    # bands by degree rank (pads sink to the end)
    order = np.argsort(-degp, kind="stable")
    rank = np.empty(NPAD, dtype=np.int64)
    rank[order] = np.arange(NPAD)
    band = rank // (NC * P)          # slot of every node

    # out-adjacency CSR over padded ids (pads have none)
    eo = np.argsort(src, kind="stable")
    s_src = src[eo]
    s_dst_by_src = dst[eo]
    out_start = np.searchsorted(s_src, np.arange(NPAD))
    out_end = np.searchsorted(s_src, np.arange(NPAD), side="right")

    is_lo = color_nodes(src, dst, band, out_start, out_end, s_dst_by_src)

    # placement: per band, lo members -> lo cells, hi -> hi cells
    node_core = np.empty(NPAD, dtype=np.int64)
    node_p = np.empty(NPAD, dtype=np.int64)
    for b in range(SLOTS):
        members = np.where(band == b)[0]
        lo_m = members[is_lo[members]]
        hi_m = members[~is_lo[members]]
        lo_cells_c = np.repeat(np.arange(5), P)
        lo_cells_p = np.tile(np.arange(P), 5)
        if b < 6:
            lo_cells_c = np.concatenate([lo_cells_c, np.full(P, 5)])
            lo_cells_p = np.concatenate([lo_cells_p, np.arange(P)])
            hi_cells_c = np.repeat(np.arange(6, 8), P)
            hi_cells_p = np.tile(np.arange(P), 2)
        else:
            hi_cells_c = np.repeat(np.arange(5, 8), P)
            hi_cells_p = np.tile(np.arange(P), 3)
        assert len(lo_m) == len(lo_cells_c) and len(hi_m) == len(hi_cells_c)
        node_core[lo_m] = lo_cells_c[:len(lo_m)]
        node_p[lo_m] = lo_cells_p[:len(lo_m)]
        node_core[hi_m] = hi_cells_c[:len(hi_m)]
        node_p[hi_m] = hi_cells_p[:len(hi_m)]
    row = node_core * CPN + band * P + node_p
    assert (is_lo == (row < ALO)).all()

    # per-dst phase counts and band templates
    srow = row[src]
    e_is_a = srow < ALO
    acnt = np.bincount(dst[e_is_a], minlength=NPAD)
    bcnt = np.bincount(dst[~e_is_a], minlength=NPAD)
    tmplA_raw = np.zeros(SLOTS, dtype=np.int64)
    tmplB_raw = np.zeros(SLOTS, dtype=np.int64)
    np.maximum.at(tmplA_raw, band, acnt)
    np.maximum.at(tmplB_raw, band, bcnt)

    def quantize(raw):
        lv = choose_levels(raw)
        out = np.zeros_like(raw)
        pos = raw > 0
        out[pos] = lv[np.searchsorted(lv, raw[pos])]
        return out

    tmplA = quantize(tmplA_raw)
    tmplB = quantize(tmplB_raw)
    offA = np.concatenate([[0], np.cumsum(tmplA)])
    offB = np.concatenate([[0], np.cumsum(tmplB)])
    TJA, TJB = int(offA[-1]), int(offB[-1])

    # queue split: contiguous band ranges with ~equal total cols
    colw = tmplA + tmplB
    cum = np.cumsum(colw)
    tot = int(cum[-1])
    bq = [0]
    for q in range(1, 4):
        bq.append(int(np.searchsorted(cum, tot * q / 4)))
    bq.append(SLOTS)

    # per-queue instruction cuts at band boundaries, <= MAXCOL cols each
    def cut_instrs(tmpl, off, b0, b1, phase):
        instrs = []
        s = b0
        while s < b1:
            e = s
            cols = 0
            while e < b1 and cols + tmpl[e] <= MAXCOL:
                cols += int(tmpl[e])
                e += 1
            if e == s:
                raise RuntimeError(f"band {s} level {tmpl[s]} > {MAXCOL}")
            if cols > 0:
                instrs.append(dict(phase=phase, b0=s, b1=e, ncols=cols,
                                   coff=int(off[s])))
            s = e
        return instrs

    queue_instrs = []
    for q in range(4):
        qi = cut_instrs(tmplA, offA, bq[q], bq[q + 1], 0) + \
             cut_instrs(tmplB, offB, bq[q], bq[q + 1], 1)
        queue_instrs.append(qi)

    # waves: w-th instr of each queue; assign idx col offsets
    nwaves = max(len(qi) for qi in queue_instrs)
    waves = []
    idxcol = 0
    for w in range(nwaves):
        wave = []
        for q in range(4):
            if w < len(queue_instrs[q]):
                ins = dict(queue_instrs[q][w])
                ins["q"] = q
                ins["sect"] = q
                ins["idxcol"] = idxcol
                idxcol += ins["ncols"] * 8
                wave.append(ins)
        waves.append(wave)
    TOTC = idxcol

    # runs per wave (for DVE reduces): within each instr, maximal equal-level
    # band ranges
    for wave in waves:
        for ins in wave:
            tmpl = tmplA if ins["phase"] == 0 else tmplB
            runs = []
            s = ins["b0"]
            scol = 0
            while s < ins["b1"]:
                lvl = int(tmpl[s])
                e = s
                while e < ins["b1"] and tmpl[e] == lvl:
                    e += 1
                if lvl > 0:
                    runs.append(dict(slot0=s, R=e - s, lvl=lvl, scol=scol))
                scol += (e - s) * lvl
                s = e
            ins["runs"] = runs

    # edge -> (phase, global col, partition) idx values
    okey = np.argsort(dst * 2 + (~e_is_a), kind="stable")
    sd = dst[okey]
    sph = (~e_is_a[okey]).astype(np.int64)
    ssrow = srow[okey]
    key = sd * 2 + sph
    run_start = np.searchsorted(key, key)
    erank = np.arange(E) - run_start

    padA = int(row[np.where((degp == 0) & (row < ALO))[0][0]])
    padB = int(row[np.where((degp == 0) & (row >= ALO))[0][0]])
    idxA = np.full((NC, TJA, P), padA, dtype=np.int32)
    idxB = np.full((NC, TJB, P), padB - ALO, dtype=np.int32)
    d_core = node_core[sd]
    d_p = node_p[sd]
    d_band = band[sd]
    a_m = sph == 0
    idxA[d_core[a_m], offA[d_band[a_m]] + erank[a_m], d_p[a_m]] = ssrow[a_m]
    idxB[d_core[~a_m], offB[d_band[~a_m]] + erank[~a_m], d_p[~a_m]] = \
        ssrow[~a_m] - ALO

    # per-core idx tensor [128, TOTC] int16 (wrapped-16, replicated x8)
    idx_cores = []
    for c in range(NC):
        blocks = []
        for wave in waves:
            for ins in wave:
                arr = idxA if ins["phase"] == 0 else idxB
                blk = arr[c, ins["coff"]:ins["coff"] + ins["ncols"], :]
                flat = blk.reshape(-1).astype(np.int16)
                wrapped = flat.reshape(-1, 16).T        # [16, ncols*8]
                blocks.append(np.tile(wrapped, (8, 1)))
        idx_cores.append(np.ascontiguousarray(
            np.concatenate(blocks, axis=1).astype(np.int16)))

    return dict(
        deg=degp, band=band, node_core=node_core, node_p=node_p, row=row,
        tmplA=tmplA, tmplB=tmplB, waves=waves, TOTC=TOTC,
        idx_cores=idx_cores, TJA=TJA, TJB=TJB,
    )


def make_host_data(feat, weight, src, dst):
    n_nodes = feat.shape[0]
    lay = build_layout(src, dst, n_nodes)

    degc = np.maximum(lay["deg"].astype(np.float64), 1.0)
    dh = (degc ** -0.5).astype(np.float32)
    dh2 = (dh * dh).astype(np.float32)
    dhinv = (1.0 / dh).astype(np.float32)

    featp = np.zeros((NPAD, D), np.float32)
    featp[:n_nodes] = feat
    g0 = featp * dh[:, None]                       # [NPAD, 32]

    row = lay["row"]
    g0pack = np.zeros((NPAD, 4 * D), np.float16)
    g0pack[row, 0:D] = g0.astype(np.float16)       # 256B rows, global order

    node_core, node_p, band = lay["node_core"], lay["node_p"], lay["band"]
    per_core = []
    for c in range(NC):
        m = node_core == c
        pp, ss = node_p[m], band[m]
        gown = np.zeros((P, SLOTS, D), np.float32)
        gown[pp, ss] = g0[m]
        dh2l = np.zeros((P, SLOTS), np.float32)
        dhhl = np.zeros((P, SLOTS), np.float32)
        dhil = np.zeros((P, SLOTS), np.float32)
        dh2l[pp, ss] = dh2[m]
        dhhl[pp, ss] = dh[m]
        dhil[pp, ss] = dhinv[m]
        per_core.append(dict(
            g0pack=g0pack,
            gown0=np.ascontiguousarray(gown.reshape(P, SLOTS * D)),
            idx=lay["idx_cores"][c],
            dh2l=np.ascontiguousarray(dh2l),
            dhhl=np.ascontiguousarray(dhhl),
            dhil=np.ascontiguousarray(dhil),
            w=np.ascontiguousarray(weight.reshape(1, 3).astype(np.float32)),
            coef=np.array([[0.25, 0.5, 0.25, 0.0, 0.5, 0.25]], np.float32),
        ))
    return lay, per_core


def assemble_output(lay, outs, n_nodes):
    res = np.empty((NPAD, D), np.float32)
    node_core, node_p, band = lay["node_core"], lay["node_p"], lay["band"]
    for c in range(NC):
        m = node_core == c
        full = outs[c].reshape(P, SLOTS, D)
        res[np.where(m)[0]] = full[node_p[m], band[m]]
    return res[:n_nodes]


# --------------------------------------------------------------------------
# device kernel
# --------------------------------------------------------------------------

def dma_gather_raw(gp, out_ap, in_ap, idxs_ap, num_idxs, queue_num):
    """dma_gather, elem 64B (32 fp16), 256B row stride, no 256B-elem assert."""
    return gp.add_instruction(
        mybir.InstDMAGatherAnt(
            name=gp.bass.get_next_instruction_name(),
            ins=[*gp.lower_ap_dma(in_ap, for_custom_bir_dma=True),
                 gp.lower_ap(idxs_ap),
                 gp.lower_val_access(gp.to_reg(num_idxs))],
            outs=[gp.lower_ap(out_ap)],
            transpose=False, num_idxs=num_idxs, elem_size=32,
            stride_bytes_256=1, gen_mode=0, single_packet=False,
            queue_num=queue_num, sbuf_tokens_per_rank=0,
            sbuf_free_dim_per_rank=0, sbuf_free_dim_pad_per_rank=0,
            sbuf_byte_offset=0,
        )
    )


def build_kernel(waves, TOTC):
    SD = SLOTS * D

    nc = bacc.Bacc("TRN2", target_bir_lowering=False, num_swdge_queues=4,
                   dynamic_dma_scratch_size=32768)
    g0pack_d = nc.dram_tensor("g0pack", [NPAD, 4 * D], F16,
                              kind="ExternalInput")
    gown0_d = nc.dram_tensor("gown0", [P, SD], F32, kind="ExternalInput")
    idx_d = nc.dram_tensor("idx", [P, TOTC], I16, kind="ExternalInput")
    dh2l_d = nc.dram_tensor("dh2l", [P, SLOTS], F32, kind="ExternalInput")
    dhhl_d = nc.dram_tensor("dhhl", [P, SLOTS], F32, kind="ExternalInput")
    dhil_d = nc.dram_tensor("dhil", [P, SLOTS], F32, kind="ExternalInput")
    w_d = nc.dram_tensor("w", [1, 3], F32, kind="ExternalInput")
    coef_d = nc.dram_tensor("coef", [1, 6], F32, kind="ExternalInput")
    out_d = nc.dram_tensor("out", [P, SD], F32, kind="ExternalOutput")

    with tile.TileContext(nc) as tc:
        with (
            tc.tile_pool(name="dram", bufs=1, space="DRAM") as dramp,
            tc.tile_pool(name="persist", bufs=1) as persist,
            tc.tile_pool(name="slabp", bufs=3) as slabp,
        ):
            T = [dramp.tile([NPAD, P], F16, name=f"T{i}") for i in range(2)]
            gtabs = [dramp.tile([P, NPAD * D // P], F16, name=f"gtab{i}",
                                addr_space="Shared") for i in range(2)]
            bounces = [dramp.tile([16, CPN * D // 16], F16, name=f"bnc{i}")
                       for i in range(2)]

            idxt = persist.tile([P, TOTC], I16, name="idxt")
            nc.sync.dma_start(out=idxt[:], in_=idx_d[:])
            dh2l = persist.tile([P, SLOTS], F32, name="dh2l")
            dhhl = persist.tile([P, SLOTS], F32, name="dhhl")
            dhil = persist.tile([P, SLOTS], F32, name="dhil")
            nc.sync.dma_start(out=dh2l[:], in_=dh2l_d[:])
            nc.sync.dma_start(out=dhhl[:], in_=dhhl_d[:])
            nc.sync.dma_start(out=dhil[:], in_=dhil_d[:])

            # scalars s1, s2 folded into dhil/dhhl (same as v2)
            wt = persist.tile([1, 3], F32, name="wt")
            coefs = persist.tile([1, 6], F32, name="coefs")
            nc.sync.dma_start(out=wt[:], in_=w_d[:])
            nc.sync.dma_start(out=coefs[:], in_=coef_d[:])
            wr = persist.tile([1, 3], F32, name="wr")
            nc.vector.tensor_scalar(out=wr[:], in0=wt[:], scalar1=0.0,
                                    scalar2=None, op0=mybir.AluOpType.max)
            sprod = persist.tile([1, 6], F32, name="sprod")
            nc.vector.tensor_tensor(out=sprod[:, 0:3], in0=wr[:],
                                    in1=coefs[:, 0:3], op=mybir.AluOpType.mult)
            nc.vector.tensor_tensor(out=sprod[:, 3:6], in0=wr[:],
                                    in1=coefs[:, 3:6], op=mybir.AluOpType.mult)
            svals = persist.tile([1, 2], F32, name="svals")
            nc.vector.tensor_reduce(out=svals[:, 0:1], in_=sprod[:, 0:3],
                                    axis=mybir.AxisListType.X,
                                    op=mybir.AluOpType.add)
            nc.vector.tensor_reduce(out=svals[:, 1:2], in_=sprod[:, 3:6],
                                    axis=mybir.AxisListType.X,
                                    op=mybir.AluOpType.add)
            sbc = persist.tile([P, 2], F32, name="sbc")
            nc.gpsimd.partition_broadcast(sbc[:], svals[:])
            nc.vector.tensor_scalar(out=dhil[:], in0=dhil[:],
                                    scalar1=sbc[:, 0:1], scalar2=None,
                                    op0=mybir.AluOpType.mult)
            nc.vector.tensor_scalar(out=dhhl[:], in0=dhhl[:],
                                    scalar1=sbc[:, 1:2], scalar2=None,
                                    op0=mybir.AluOpType.mult)

            zt = persist.tile([P, 1], F32, name="zt")
            nc.gpsimd.memset(zt[:], 0.0)

            gcur = persist.tile([P, SD], F32, name="gcur")
            nc.sync.dma_start(out=gcur[:], in_=gown0_d[:])
            agg = persist.tile([P, SD], F32, name="agg")
            aggB = persist.tile([P, SD], F32, name="aggB")
            tt = persist.tile([P, SD], F32, name="tt")
            ut = persist.tile([P, SD], F32, name="ut")
            shipt = [persist.tile([P, SD], F16, name=f"shipt{i}")
                     for i in range(2)]

            for k in (1, 2, 3):
                Tk = g0pack_d if k == 1 else T[k % 2]
                nc.gpsimd.memset(agg[:], 0.0)
                nc.gpsimd.memset(aggB[:], 0.0)
                for wv, wave in enumerate(waves):
                    slabs = {}
                    for ins in wave:
                        slab = slabp.tile([P, MAXCOL * D], F16,
                                          tag=f"slab{ins['q']}",
                                          name=f"slab{k}_{wv}_{ins['q']}")
                        slabs[ins["q"]] = slab
                        in_ap = Tk[0:ALO, 0:D] if ins["phase"] == 0 \
                            else Tk[ALO:NPAD, 0:D]
                        dma_gather_raw(
                            nc.gpsimd,
                            out_ap=slab[:, 0:ins["ncols"] * D]
                            .rearrange("p (c d) -> p c d", d=D),
                            in_ap=in_ap,
                            idxs_ap=idxt[:, ins["idxcol"]:
                                         ins["idxcol"] + ins["ncols"] * 8],
                            num_idxs=ins["ncols"] * P, queue_num=ins["q"])
                    for ins in wave:
                        dst_t = agg if ins["phase"] == 0 else aggB
                        slab = slabs[ins["q"]]
                        for r in ins["runs"]:
                            s0, R, b = r["slot0"], r["R"], r["lvl"]
                            o = r["scol"] * D
                            nc.vector.tensor_reduce(
                                out=dst_t[:, s0 * D:(s0 + R) * D]
                                .rearrange("p (r j) -> p r j", j=D),
                                in_=slab[:, o:o + R * b * D]
                                .rearrange("p (r b j) -> p r j b", b=b, j=D),
                                axis=mybir.AxisListType.X,
                                op=mybir.AluOpType.add)
                nc.vector.tensor_tensor(out=agg[:], in0=agg[:], in1=aggB[:],
                                        op=mybir.AluOpType.add)
                if k < 3:
                    # g_k = g_{k-1} + dh^2 * agg
                    nc.vector.tensor_tensor(
                        out=tt[:].rearrange("p (s j) -> p s j", j=D),
                        in0=agg[:].rearrange("p (s j) -> p s j", j=D),
                        in1=dh2l[:].to_broadcast([P, SLOTS, D]),
                        op=mybir.AluOpType.mult)
                    nc.vector.tensor_tensor(out=gcur[:], in0=gcur[:],
                                            in1=tt[:],
                                            op=mybir.AluOpType.add)
                    sh = shipt[k - 1]
                    nc.vector.tensor_tensor(
                        out=sh[:], in0=gcur[:],
                        in1=zt[:].to_broadcast([P, SD]),
                        op=mybir.AluOpType.add)
                    # ship in (slot, p, dim) order so AllGather concat is the
                    # global row order
                    nc.sync.dma_start(
                        out=bounces[k - 1][:]
                        .rearrange("a b -> (a b)")
                        .rearrange("(s p j) -> p s j", s=SLOTS, p=P),
                        in_=sh[:].rearrange("p (s j) -> p s j", j=D))
                    nc.gpsimd.collective_compute(
                        "AllGather", mybir.AluOpType.bypass,
                        replica_groups=[list(range(NC))],
                        ins=[bounces[k - 1].opt()],
                        outs=[gtabs[k - 1].opt()])
                    # respace into next round's table
                    nc.sync.dma_start(
                        out=T[(k + 1) % 2][:, 0:D],
                        in_=gtabs[k - 1][:].rearrange("a b -> (a b)")
                        .rearrange("(r j) -> r j", j=D))
                else:
                    # out = s1*dhil*g2 - s2*dhhl*agg (s1, s2 pre-folded)
                    nc.vector.tensor_tensor(
                        out=ut[:].rearrange("p (s j) -> p s j", j=D),
                        in0=gcur[:].rearrange("p (s j) -> p s j", j=D),
                        in1=dhil[:].to_broadcast([P, SLOTS, D]),
                        op=mybir.AluOpType.mult)
                    nc.vector.tensor_tensor(
                        out=tt[:].rearrange("p (s j) -> p s j", j=D),
                        in0=agg[:].rearrange("p (s j) -> p s j", j=D),
                        in1=dhhl[:].to_broadcast([P, SLOTS, D]),
                        op=mybir.AluOpType.mult)
                    nc.vector.tensor_tensor(out=ut[:], in0=ut[:], in1=tt[:],
                                            op=mybir.AluOpType.subtract)
                    nc.sync.dma_start(out=out_d[:], in_=ut[:])
    nc.compile()
    return nc


_CACHE = {}


def _plan_key(waves, TOTC):
    key = [TOTC]
    for wave in waves:
        for ins in wave:
            key.append((ins["q"], ins["phase"], ins["b0"], ins["b1"],
                        ins["ncols"], ins["coff"], ins["idxcol"],
                        tuple((r["slot0"], r["R"], r["lvl"], r["scol"])
                              for r in ins["runs"])))
        key.append(None)
    return tuple(key)


def kernel(feat, weight, src, dst):
    feat = np.ascontiguousarray(np.asarray(feat, dtype=np.float32))
    weight = np.ascontiguousarray(np.asarray(weight, dtype=np.float32))
    src64 = np.asarray(src).astype(np.int64)
    dst64 = np.asarray(dst).astype(np.int64)
    n_nodes = feat.shape[0]

    lay, per_core = make_host_data(feat, weight, src64, dst64)
    key = _plan_key(lay["waves"], lay["TOTC"])
    if key not in _CACHE:
        _CACHE[key] = build_kernel(lay["waves"], lay["TOTC"])
    nc = _CACHE[key]

    keys = ["g0pack", "gown0", "idx", "dh2l", "dhhl", "dhil", "w", "coef"]
    in_maps = [{k: pc[k] for k in keys} for pc in per_core]
    res = bass_utils.run_bass_kernel_spmd(nc, in_maps, core_ids=list(range(NC)))
    outs = [res.results[c]["out"] for c in range(NC)]
    return assemble_output(lay, outs, n_nodes)
